# revision 6
# baseline (speedup 1.0000x reference)
"""Contrastive (NT-Xent) loss kernel for Trainium2, 8 NeuronCores SPMD.

Math (B=4096, D=256, T=0.5):
  z = l2norm(emb) rows; reps=[z_i; z_j] (8192 x 256); sim = reps @ reps.T
  denom_r = sum_{c != r} exp(sim[r,c]/T);  pos_m = z_i[m].z_j[m]
  loss = mean_r( ln(denom_r) - pos_r/T )

Distribution (per sharding hint): core k receives ONLY its row shard
(emb_i rows [512k,512k+512) and emb_j rows likewise, fp16). It normalizes
its own 1024 reps rows, transposes them to d-major, AllGathers the
transposed fp16 reps across the 8 cores on-device (4MB), computes its
1024-row block of exp(sim/T) row-sums, and AllReduces the scalar-partial
[128,1] so every core holds the full-batch partial. The host fetches a
single 512B shard. Column order after the gather is a permutation of the
reference's reps order; row-wise denominators are permutation-invariant.

Per-core pipeline:
  - load own xa/xb fp16 [512,256] -> [128,4,256] tiles
  - rowwise sq-sums (DVE), inv_norm = Exp(-0.5*Ln(s)) (ACT), z = x*inv (fp16)
  - positives pos = (xa.xb)*inv_a*inv_b
  - DMA-xbar transpose own z -> zT halves [128d, 1024cols], store to DRAM
  - AllGather zT (fp16, 512KB->4MB) across 8 cores
  - per 2048-col group g: load rhs from gathered DRAM; per m-tile: matmul
    fp16 -> PSUM fp32 [128,2048], ACT Exp(scale=2) with accum_out row-sums
  - ln(rowsum - e^2) - 4*pos -> partial [128,1]; AllReduce add -> out
Host: loss = out_shard0.sum()/(2B).
"""

import os
import numpy as np
from contextlib import ExitStack

import concourse.bass as bass
import concourse.tile as tile
from concourse import bacc, mybir

B = 4096
D = 256
TEMP = 0.5
NCORES = 8
ROWS = 2 * B            # 8192 reps rows
PER = B // NCORES       # 512 rows of emb_i (and emb_j) per core
OWN = 2 * PER           # 1024 reps rows per core
P = 128
NG = 4                  # column groups
GCOLS = ROWS // NG      # 2048 columns per group
MT = OWN // P           # 8 m-tiles per core
F32 = mybir.dt.float32
DT = mybir.dt.float16   # wire + matmul dtype
INV_T = 1.0 / TEMP      # 2.0
DIAG = float(np.exp(np.float32(INV_T), dtype=np.float32))  # exp(2*||z||^2), ||z||~1


def _kernel_body(ctx: ExitStack, tc: tile.TileContext, out_ap, xa, xb):
    nc = tc.nc
    AF = mybir.ActivationFunctionType
    ALU = mybir.AluOpType

    own_pool = ctx.enter_context(tc.tile_pool(name="own", bufs=1))
    sq_pool = ctx.enter_context(tc.tile_pool(name="sq", bufs=2))
    zt_pool = ctx.enter_context(tc.tile_pool(name="zt", bufs=1))
    fin_pool = ctx.enter_context(tc.tile_pool(name="fin", bufs=1))
    ps_pool = ctx.enter_context(tc.tile_pool(name="ps", bufs=2, space="PSUM"))
    dram = ctx.enter_context(tc.tile_pool(name="dram", bufs=1, space="DRAM"))

    rowparts = fin_pool.tile([P, MT * NG], F32, tag="rowparts")
    negdiag = fin_pool.tile([P, 1], F32, tag="negdiag")
    nc.gpsimd.memset(negdiag[:], -DIAG)

    # ---------------- own-block prologue ----------------
    nt_own = PER // P  # 4
    own_x = own_pool.tile([P, 2 * nt_own, D], DT, tag="own_x")  # [128,8,256] fp16
    nc.sync.dma_start(own_x[:, 0:nt_own, :], xa.rearrange("(t p) d -> p t d", p=P))
    nc.sync.dma_start(own_x[:, nt_own:2 * nt_own, :], xb.rearrange("(t p) d -> p t d", p=P))

    sq3 = sq_pool.tile([P, 2 * nt_own, D], F32, tag="sq3", name="sq3")
    nc.vector.tensor_mul(sq3[:], own_x[:], own_x[:])
    sqs = own_pool.tile([P, 2 * nt_own], F32, tag="sqs")
    nc.vector.reduce_sum(out=sqs[:], in_=sq3[:], axis=mybir.AxisListType.X)
    inv = own_pool.tile([P, 2 * nt_own], F32, tag="inv")
    nc.scalar.activation(out=inv[:], in_=sqs[:], func=AF.Ln)
    nc.scalar.activation(out=inv[:], in_=inv[:], func=AF.Exp, scale=-0.5)

    z_own = own_pool.tile([P, 2 * nt_own, D], DT, tag="z_own")
    for t in range(2 * nt_own):
        nc.vector.tensor_scalar_mul(
            out=z_own[:, t, :], in0=own_x[:, t, :], scalar1=inv[:, t:t + 1])

    # positives: pos_t = (xa[t] . xb[t]) * inv_a[t] * inv_b[t]
    pr3 = sq_pool.tile([P, nt_own, D], F32, tag="sq3", name="pr3")
    nc.vector.tensor_mul(pr3[:], own_x[:, 0:nt_own, :], own_x[:, nt_own:2 * nt_own, :])
    pos_raw = own_pool.tile([P, nt_own], F32, tag="pos_raw")
    nc.vector.reduce_sum(out=pos_raw[:], in_=pr3[:], axis=mybir.AxisListType.X)
    pos = own_pool.tile([P, nt_own], F32, tag="pos")
    nc.vector.tensor_mul(pos[:], pos_raw[:], inv[:, 0:nt_own])
    nc.vector.tensor_mul(pos[:], pos[:], inv[:, nt_own:2 * nt_own])

    # transpose own z to d-major halves: zt_own[h][d, c] = z_own[c, :, h*128+d]
    zt_own = [own_pool.tile([P, OWN], DT, tag=f"zt_own{h}", name=f"zt_own{h}")
              for h in range(2)]
    for h in range(2):
        for t in range(2 * nt_own):
            nc.sync.dma_start_transpose(
                out=zt_own[h][:, t * P:(t + 1) * P],
                in_=z_own[:, t, h * P:(h + 1) * P])

    # ---------------- gather reps across cores ----------------
    ccin = dram.tile([2 * P, OWN], DT, tag="ccin", name="ccin")       # [256,1024]
    nc.sync.dma_start(ccin[0:P, :], zt_own[0][:])
    nc.sync.dma_start(ccin[P:2 * P, :], zt_own[1][:])
    ccout = dram.tile([NCORES * 2 * P, OWN], DT, tag="ccout", name="ccout")
    nc.gpsimd.collective_compute(
        "AllGather", ALU.bypass,
        replica_groups=[list(range(NCORES))],
        ins=[ccin[:].opt()], outs=[ccout[:].opt()])

    # rhs tiles: group g covers gathered cols of ranks 2g, 2g+1
    zt = [[None, None] for _ in range(NG)]
    for g in range(NG):
        for h in range(2):
            ztg = zt_pool.tile([P, GCOLS], DT, tag=f"zt{g}_{h}", name=f"zt{g}_{h}")
            for u in range(2):
                r = 2 * g + u
                nc.sync.dma_start(
                    ztg[:, u * OWN:(u + 1) * OWN],
                    ccout[r * 2 * P + h * P: r * 2 * P + (h + 1) * P, :])
            zt[g][h] = ztg

    # ---------------- main matmul + exp row-sums ----------------
    for g in range(NG):
        for m in range(MT):
            ps = ps_pool.tile([P, GCOLS], F32, tag="ps")
            nsub = GCOLS // 512
            for ns in range(nsub):
                nc.tensor.matmul(
                    ps[:, ns * 512:(ns + 1) * 512],
                    lhsT=zt_own[0][:, m * P:(m + 1) * P],
                    rhs=zt[g][0][:, ns * 512:(ns + 1) * 512],
                    start=True, stop=False)
            for ns in range(nsub):
                nc.tensor.matmul(
                    ps[:, ns * 512:(ns + 1) * 512],
                    lhsT=zt_own[1][:, m * P:(m + 1) * P],
                    rhs=zt[g][1][:, ns * 512:(ns + 1) * 512],
                    start=False, stop=True)
            nc.scalar.activation(
                out=ps[:], in_=ps[:], func=AF.Exp, scale=INV_T,
                accum_out=rowparts[:, m * NG + g: m * NG + g + 1])

    # ---------------- tail ----------------
    denom = fin_pool.tile([P, MT], F32, tag="denom")
    nc.vector.reduce_sum(
        out=denom[:], in_=rowparts[:].rearrange("p (m g) -> p m g", g=NG),
        axis=mybir.AxisListType.X)
    ln8 = fin_pool.tile([P, MT], F32, tag="ln8")
    nc.scalar.activation(out=ln8[:], in_=denom[:], func=AF.Ln, bias=negdiag[:])
    lnsum = fin_pool.tile([P, 1], F32, tag="lnsum")
    nc.vector.reduce_sum(out=lnsum[:], in_=ln8[:], axis=mybir.AxisListType.X)
    possum = fin_pool.tile([P, 1], F32, tag="possum")
    nc.vector.reduce_sum(out=possum[:], in_=pos[:], axis=mybir.AxisListType.X)
    partial = fin_pool.tile([P, 1], F32, tag="partial")
    # partial = lnsum - 2*INV_T*possum   (each pos appears for a z_i and a z_j row)
    nc.vector.tensor_scalar(
        out=partial[:], in0=possum[:], scalar1=-2.0 * INV_T, scalar2=lnsum[:],
        op0=ALU.mult, op1=ALU.add)

    # all-reduce the per-core partial so any single shard is the full answer
    ar_in = dram.tile([P, 1], F32, tag="ar_in", name="ar_in")
    ar_out = dram.tile([P, 1], F32, tag="ar_out", name="ar_out")
    nc.sync.dma_start(ar_in[:], partial[:])
    nc.gpsimd.collective_compute(
        "AllReduce", ALU.add,
        replica_groups=[list(range(NCORES))],
        ins=[ar_in[:].opt()], outs=[ar_out[:].opt()])
    nc.gpsimd.dma_start(out_ap, ar_out[:])


_NC_CACHE = {}


def build_nc():
    if "nc" in _NC_CACHE:
        return _NC_CACHE["nc"]
    nc = bacc.Bacc("TRN2", target_bir_lowering=False, debug=False,
                   enable_asserts=False, num_devices=NCORES)
    xa = nc.dram_tensor("xa", (PER, D), DT, kind="ExternalInput").ap()
    xb = nc.dram_tensor("xb", (PER, D), DT, kind="ExternalInput").ap()
    out = nc.dram_tensor("out", (P, 1), F32, kind="ExternalOutput").ap()
    with tile.TileContext(nc) as tc:
        with ExitStack() as ctx:
            _kernel_body(ctx, tc, out, xa, xb)
    nc.compile()
    _NC_CACHE["nc"] = nc
    return nc


def make_in_maps(emb_i16, emb_j16):
    return [{"xa": emb_i16[k * PER:(k + 1) * PER],
             "xb": emb_j16[k * PER:(k + 1) * PER]} for k in range(NCORES)]


# ---------------- cached PJRT dispatcher ----------------
# run_bass_kernel_spmd rebuilds jit(shard_map(...)) on every call (fresh
# closure -> jit cache miss -> full retrace each run). Build it once and
# reuse; identical execution path (same _bass_exec_p custom call, same NEFF,
# cores 0-7), minus the per-call retrace.

_DISP = {}


def _dispatcher():
    if "d" in _DISP:
        return _DISP["d"]
    import jax
    from jax.sharding import Mesh, PartitionSpec
    try:
        from jax.experimental.shard_map import shard_map  # what bass2jax uses
        sm_kw = {"check_rep": False}
    except ImportError:
        from jax import shard_map
        sm_kw = {"check_vma": False}
    from concourse.bass2jax import (
        _bass_exec_p, install_neuronx_cc_hook, partition_id_tensor)

    nc = build_nc()
    install_neuronx_cc_hook()

    partition_name = nc.partition_id_tensor.name if nc.partition_id_tensor else None
    in_names, out_names, out_avals, zero_shapes = [], [], [], []
    for alloc in nc.m.functions[0].allocations:
        if not isinstance(alloc, mybir.MemoryLocationSet):
            continue
        name = alloc.memorylocations[0].name
        if alloc.kind == "ExternalInput":
            if name != partition_name:
                in_names.append(name)
        elif alloc.kind == "ExternalOutput":
            shape = tuple(alloc.tensor_shape)
            dtype = mybir.dt.np(alloc.dtype)
            out_names.append(name)
            out_avals.append(jax.core.ShapedArray(shape, dtype))
            zero_shapes.append((shape, dtype))
    n_params = len(in_names)
    n_outs = len(out_names)
    in_names_all = list(in_names) + list(out_names)
    if partition_name is not None:
        in_names_all.append(partition_name)
    donate = tuple(range(n_params, n_params + n_outs))

    def _body(*args):
        operands = list(args)
        if partition_name is not None:
            operands.append(partition_id_tensor())
        outs = _bass_exec_p.bind(
            *operands,
            out_avals=tuple(out_avals),
            in_names=tuple(in_names_all),
            out_names=tuple(out_names),
            lowering_input_output_aliases=(),
            sim_require_finite=True,
            sim_require_nnan=True,
            nc=nc,
        )
        return tuple(outs)

    devices = jax.devices()[:NCORES]
    assert len(devices) == NCORES
    mesh = Mesh(np.asarray(devices), ("core",))
    in_specs = (PartitionSpec("core"),) * (n_params + n_outs)
    out_specs = (PartitionSpec("core"),) * n_outs
    sharded = jax.jit(
        shard_map(_body, mesh=mesh, in_specs=in_specs, out_specs=out_specs,
                  **sm_kw),
        donate_argnums=donate, keep_unused=True)
    d = {"sharded": sharded, "in_names": in_names,
         "out_names": out_names, "zero_shapes": zero_shapes}
    _DISP["d"] = d
    return d


def run_cached(emb_i16, emb_j16):
    """One SPMD run via the cached dispatcher; returns core-0's out shard."""
    d = _dispatcher()
    glob = {"xa": emb_i16, "xb": emb_j16}  # axis-0 concat of per-core shards
    args = [glob[name] for name in d["in_names"]]
    zeros = [np.zeros((NCORES * s[0], *s[1:]), dt) for s, dt in d["zero_shapes"]]
    out_arrs = d["sharded"](*args, *zeros)
    try:
        # single-shard fetch: out is AllReduced, any core's [128,1] is the answer
        return np.asarray(out_arrs[0].addressable_shards[0].data)
    except Exception:
        return np.asarray(out_arrs[0])[:P]


def run_spmd(emb_i16, emb_j16):
    """Fallback: same NEFF via bass_utils.run_bass_kernel_spmd."""
    from concourse import bass_utils
    nc = build_nc()
    res = bass_utils.run_bass_kernel_spmd(
        nc, make_in_maps(emb_i16, emb_j16), core_ids=list(range(NCORES)))
    return np.asarray(res.results[0]["out"])


def kernel(emb_i, emb_j):
    emb_i16 = np.ascontiguousarray(np.asarray(emb_i)).astype(np.float16)
    emb_j16 = np.ascontiguousarray(np.asarray(emb_j)).astype(np.float16)
    if os.environ.get("CL_DISPATCH", "cached") == "spmd":
        part = run_spmd(emb_i16, emb_j16)
    else:
        part = run_cached(emb_i16, emb_j16)
    loss = np.float32(part.astype(np.float64).sum() / ROWS)
    return np.asarray(loss, dtype=np.float32)


# revision 11
# speedup vs baseline: 1.1925x; 1.1925x over previous
"""Contrastive (NT-Xent) loss kernel for Trainium2, 8 NeuronCores SPMD.

Math (B=4096, D=256, T=0.5):
  z = l2norm(emb) rows; reps=[z_i; z_j] (8192 x 256); sim = reps @ reps.T
  denom_r = sum_{c != r} exp(sim[r,c]/T);  pos_m = z_i[m].z_j[m]
  loss = mean_r( ln(denom_r) - pos_r/T )

Distribution (per sharding hint): core k receives ONLY its row shard
(emb_i rows [512k,512k+512) and emb_j rows likewise, fp16). It normalizes
its own 1024 reps rows, transposes them to d-major, AllGathers the
transposed fp16 reps across the 8 cores on-device (4MB), computes its
1024-row block of exp(sim/T) row-sums, and AllReduces the scalar-partial
[128,1] so every core holds the full-batch partial. The host fetches a
single 512B shard. Column order after the gather is a permutation of the
reference's reps order; row-wise denominators are permutation-invariant.

Per-core pipeline:
  - load own xa/xb fp16 [512,256] -> [128,4,256] tiles
  - rowwise sq-sums (DVE), inv_norm = Exp(-0.5*Ln(s)) (ACT), z = x*inv (fp16)
  - positives pos = (xa.xb)*inv_a*inv_b
  - DMA-xbar transpose own z -> zT halves [128d, 1024cols], store to DRAM
  - AllGather zT (fp16, 512KB->4MB) across 8 cores
  - per 2048-col group g: load rhs from gathered DRAM; per m-tile: matmul
    fp16 -> PSUM fp32 [128,2048], ACT Exp(scale=2) with accum_out row-sums
  - ln(rowsum - e^2) - 4*pos -> partial [128,1]; AllReduce add -> out
Host: loss = out_shard0.sum()/(2B).
"""

import os
import numpy as np
from contextlib import ExitStack

import concourse.bass as bass
import concourse.tile as tile
from concourse import bacc, mybir

B = 4096
D = 256
TEMP = 0.5
NCORES = 8
ROWS = 2 * B            # 8192 reps rows
PER = B // NCORES       # 512 rows of emb_i (and emb_j) per core
OWN = 2 * PER           # 1024 reps rows per core
P = 128
NG = 4                  # column groups
GCOLS = ROWS // NG      # 2048 columns per group
MT = OWN // P           # 8 m-tiles per core
F32 = mybir.dt.float32
DT = mybir.dt.float16   # wire + matmul dtype
INV_T = 1.0 / TEMP      # 2.0
DIAG = float(np.exp(np.float32(INV_T), dtype=np.float32))  # exp(2*||z||^2), ||z||~1


def _kernel_body(ctx: ExitStack, tc: tile.TileContext, out_ap, x):
    nc = tc.nc
    AF = mybir.ActivationFunctionType
    ALU = mybir.AluOpType

    own_pool = ctx.enter_context(tc.tile_pool(name="own", bufs=1))
    sq_pool = ctx.enter_context(tc.tile_pool(name="sq", bufs=2))
    zt_pool = ctx.enter_context(tc.tile_pool(name="zt", bufs=1))
    fin_pool = ctx.enter_context(tc.tile_pool(name="fin", bufs=1))
    ps_pool = ctx.enter_context(tc.tile_pool(name="ps", bufs=2, space="PSUM"))
    dram = ctx.enter_context(tc.tile_pool(name="dram", bufs=1, space="DRAM"))

    rowparts = fin_pool.tile([P, MT * NG], F32, tag="rowparts")
    negdiag = fin_pool.tile([P, 1], F32, tag="negdiag")
    nc.gpsimd.memset(negdiag[:], -DIAG)

    # ---------------- own-block prologue ----------------
    # x rows: [own emb_i rows (512); own emb_j rows (512)] -> tiles 0-3 / 4-7
    nt_own = PER // P  # 4
    own_x = own_pool.tile([P, 2 * nt_own, D], DT, tag="own_x")  # [128,8,256] fp16
    nc.sync.dma_start(own_x[:], x.rearrange("(t p) d -> p t d", p=P))

    sq3 = sq_pool.tile([P, 2 * nt_own, D], F32, tag="sq3", name="sq3")
    nc.vector.tensor_mul(sq3[:], own_x[:], own_x[:])
    sqs = own_pool.tile([P, 2 * nt_own], F32, tag="sqs")
    nc.vector.reduce_sum(out=sqs[:], in_=sq3[:], axis=mybir.AxisListType.X)
    inv = own_pool.tile([P, 2 * nt_own], F32, tag="inv")
    nc.scalar.activation(out=inv[:], in_=sqs[:], func=AF.Ln)
    nc.scalar.activation(out=inv[:], in_=inv[:], func=AF.Exp, scale=-0.5)

    z_own = own_pool.tile([P, 2 * nt_own, D], DT, tag="z_own")
    for t in range(2 * nt_own):
        nc.vector.tensor_scalar_mul(
            out=z_own[:, t, :], in0=own_x[:, t, :], scalar1=inv[:, t:t + 1])

    # positives: pos_t = (xa[t] . xb[t]) * inv_a[t] * inv_b[t]
    pr3 = sq_pool.tile([P, nt_own, D], F32, tag="sq3", name="pr3")
    nc.vector.tensor_mul(pr3[:], own_x[:, 0:nt_own, :], own_x[:, nt_own:2 * nt_own, :])
    pos_raw = own_pool.tile([P, nt_own], F32, tag="pos_raw")
    nc.vector.reduce_sum(out=pos_raw[:], in_=pr3[:], axis=mybir.AxisListType.X)
    pos = own_pool.tile([P, nt_own], F32, tag="pos")
    nc.vector.tensor_mul(pos[:], pos_raw[:], inv[:, 0:nt_own])
    nc.vector.tensor_mul(pos[:], pos[:], inv[:, nt_own:2 * nt_own])

    # transpose own z to d-major halves: zt_own[h][d, c] = z_own[c, :, h*128+d]
    zt_own = [own_pool.tile([P, OWN], DT, tag=f"zt_own{h}", name=f"zt_own{h}")
              for h in range(2)]
    for h in range(2):
        for t in range(2 * nt_own):
            nc.sync.dma_start_transpose(
                out=zt_own[h][:, t * P:(t + 1) * P],
                in_=z_own[:, t, h * P:(h + 1) * P])

    # ---------------- gather reps across cores ----------------
    ccin = dram.tile([2 * P, OWN], DT, tag="ccin", name="ccin")       # [256,1024]
    nc.sync.dma_start(ccin[0:P, :], zt_own[0][:])
    nc.sync.dma_start(ccin[P:2 * P, :], zt_own[1][:])
    ccout = dram.tile([NCORES * 2 * P, OWN], DT, tag="ccout", name="ccout")
    nc.gpsimd.collective_compute(
        "AllGather", ALU.bypass,
        replica_groups=[list(range(NCORES))],
        ins=[ccin[:].opt()], outs=[ccout[:].opt()])

    # rhs tiles: group g covers gathered cols of ranks 2g, 2g+1
    zt = [[None, None] for _ in range(NG)]
    for g in range(NG):
        for h in range(2):
            ztg = zt_pool.tile([P, GCOLS], DT, tag=f"zt{g}_{h}", name=f"zt{g}_{h}")
            for u in range(2):
                r = 2 * g + u
                nc.sync.dma_start(
                    ztg[:, u * OWN:(u + 1) * OWN],
                    ccout[r * 2 * P + h * P: r * 2 * P + (h + 1) * P, :])
            zt[g][h] = ztg

    # ---------------- main matmul + exp row-sums ----------------
    for g in range(NG):
        for m in range(MT):
            ps = ps_pool.tile([P, GCOLS], F32, tag="ps")
            nsub = GCOLS // 512
            for ns in range(nsub):
                nc.tensor.matmul(
                    ps[:, ns * 512:(ns + 1) * 512],
                    lhsT=zt_own[0][:, m * P:(m + 1) * P],
                    rhs=zt[g][0][:, ns * 512:(ns + 1) * 512],
                    start=True, stop=False)
            for ns in range(nsub):
                nc.tensor.matmul(
                    ps[:, ns * 512:(ns + 1) * 512],
                    lhsT=zt_own[1][:, m * P:(m + 1) * P],
                    rhs=zt[g][1][:, ns * 512:(ns + 1) * 512],
                    start=False, stop=True)
            nc.scalar.activation(
                out=ps[:], in_=ps[:], func=AF.Exp, scale=INV_T,
                accum_out=rowparts[:, m * NG + g: m * NG + g + 1])

    # ---------------- tail ----------------
    denom = fin_pool.tile([P, MT], F32, tag="denom")
    nc.vector.reduce_sum(
        out=denom[:], in_=rowparts[:].rearrange("p (m g) -> p m g", g=NG),
        axis=mybir.AxisListType.X)
    ln8 = fin_pool.tile([P, MT], F32, tag="ln8")
    nc.scalar.activation(out=ln8[:], in_=denom[:], func=AF.Ln, bias=negdiag[:])
    lnsum = fin_pool.tile([P, 1], F32, tag="lnsum")
    nc.vector.reduce_sum(out=lnsum[:], in_=ln8[:], axis=mybir.AxisListType.X)
    possum = fin_pool.tile([P, 1], F32, tag="possum")
    nc.vector.reduce_sum(out=possum[:], in_=pos[:], axis=mybir.AxisListType.X)
    partial = fin_pool.tile([P, 1], F32, tag="partial")
    # partial = lnsum - 2*INV_T*possum   (each pos appears for a z_i and a z_j row)
    nc.vector.tensor_scalar(
        out=partial[:], in0=possum[:], scalar1=-2.0 * INV_T, scalar2=lnsum[:],
        op0=ALU.mult, op1=ALU.add)

    # all-reduce the per-core partial so any single shard is the full answer
    ar_in = dram.tile([P, 1], F32, tag="ar_in", name="ar_in")
    ar_out = dram.tile([P, 1], F32, tag="ar_out", name="ar_out")
    nc.sync.dma_start(ar_in[:], partial[:])
    nc.gpsimd.collective_compute(
        "AllReduce", ALU.add,
        replica_groups=[list(range(NCORES))],
        ins=[ar_in[:].opt()], outs=[ar_out[:].opt()])
    nc.gpsimd.dma_start(out_ap, ar_out[:])


_NC_CACHE = {}


def build_nc():
    if "nc" in _NC_CACHE:
        return _NC_CACHE["nc"]
    nc = bacc.Bacc("TRN2", target_bir_lowering=False, debug=False,
                   enable_asserts=False, num_devices=NCORES)
    x = nc.dram_tensor("x", (OWN, D), DT, kind="ExternalInput").ap()
    out = nc.dram_tensor("out", (P, 1), F32, kind="ExternalOutput").ap()
    with tile.TileContext(nc) as tc:
        with ExitStack() as ctx:
            _kernel_body(ctx, tc, out, x)
    nc.compile()
    _NC_CACHE["nc"] = nc
    return nc


def pack_inputs(emb_i, emb_j):
    """[8192,256] fp16: per core k, its 512 emb_i rows then its 512 emb_j rows."""
    combined = np.empty((NCORES, 2, PER, D), np.float16)
    np.copyto(combined[:, 0], np.asarray(emb_i).reshape(NCORES, PER, D),
              casting="unsafe")
    np.copyto(combined[:, 1], np.asarray(emb_j).reshape(NCORES, PER, D),
              casting="unsafe")
    return combined.reshape(ROWS, D)


def make_in_maps(x_global):
    return [{"x": x_global[k * OWN:(k + 1) * OWN]} for k in range(NCORES)]


# ---------------- cached PJRT dispatcher ----------------
# run_bass_kernel_spmd rebuilds jit(shard_map(...)) on every call (fresh
# closure -> jit cache miss -> full retrace each run). Build it once and
# reuse; identical execution path (same _bass_exec_p custom call, same NEFF,
# cores 0-7), minus the per-call retrace.

_DISP = {}


def _dispatcher():
    if "d" in _DISP:
        return _DISP["d"]
    import jax
    from jax.sharding import Mesh, PartitionSpec
    try:
        from jax.experimental.shard_map import shard_map  # what bass2jax uses
        sm_kw = {"check_rep": False}
    except ImportError:
        from jax import shard_map
        sm_kw = {"check_vma": False}
    from concourse.bass2jax import (
        _bass_exec_p, install_neuronx_cc_hook, partition_id_tensor)

    nc = build_nc()
    install_neuronx_cc_hook()

    partition_name = nc.partition_id_tensor.name if nc.partition_id_tensor else None
    in_names, out_names, out_avals = [], [], []
    for alloc in nc.m.functions[0].allocations:
        if not isinstance(alloc, mybir.MemoryLocationSet):
            continue
        name = alloc.memorylocations[0].name
        if alloc.kind == "ExternalInput":
            if name != partition_name:
                in_names.append(name)
        elif alloc.kind == "ExternalOutput":
            shape = tuple(alloc.tensor_shape)
            dtype = mybir.dt.np(alloc.dtype)
            out_names.append(name)
            out_avals.append(jax.core.ShapedArray(shape, dtype))
    n_params = len(in_names)
    n_outs = len(out_names)
    # No donated zero-output operands: this kernel writes every element of
    # every ExternalOutput, so uninitialized PJRT-allocated results are fine.
    in_names_all = list(in_names)
    if partition_name is not None:
        in_names_all.append(partition_name)

    def _body(*args):
        operands = list(args)
        if partition_name is not None:
            operands.append(partition_id_tensor())
        outs = _bass_exec_p.bind(
            *operands,
            out_avals=tuple(out_avals),
            in_names=tuple(in_names_all),
            out_names=tuple(out_names),
            lowering_input_output_aliases=(),
            sim_require_finite=True,
            sim_require_nnan=True,
            nc=nc,
        )
        return tuple(outs)

    devices = jax.devices()[:NCORES]
    assert len(devices) == NCORES
    mesh = Mesh(np.asarray(devices), ("core",))
    in_specs = (PartitionSpec("core"),) * n_params
    out_specs = (PartitionSpec("core"),) * n_outs
    sharded = jax.jit(
        shard_map(_body, mesh=mesh, in_specs=in_specs, out_specs=out_specs,
                  **sm_kw))
    d = {"sharded": sharded, "in_names": in_names, "out_names": out_names}
    _DISP["d"] = d
    return d


def run_cached(x_global):
    """One SPMD run via the cached dispatcher; returns core-0's out shard."""
    d = _dispatcher()
    out_arrs = d["sharded"](x_global)
    try:
        # single-shard fetch: out is AllReduced, any core's [128,1] is the answer
        return np.asarray(out_arrs[0].addressable_shards[0].data)
    except Exception:
        return np.asarray(out_arrs[0])[:P]


def run_spmd(x_global):
    """Fallback: same NEFF via bass_utils.run_bass_kernel_spmd."""
    from concourse import bass_utils
    nc = build_nc()
    res = bass_utils.run_bass_kernel_spmd(
        nc, make_in_maps(x_global), core_ids=list(range(NCORES)))
    return np.asarray(res.results[0]["out"])


def kernel(emb_i, emb_j):
    x_global = pack_inputs(emb_i, emb_j)
    if os.environ.get("CL_DISPATCH", "cached") == "spmd":
        part = run_spmd(x_global)
    else:
        part = run_cached(x_global)
    loss = np.float32(part.astype(np.float64).sum() / ROWS)
    return np.asarray(loss, dtype=np.float32)


# revision 15
# speedup vs baseline: 1.5088x; 1.2652x over previous
"""Contrastive (NT-Xent) loss kernel for Trainium2, 8 NeuronCores SPMD.

Math (B=4096, D=256, T=0.5):
  z = l2norm(emb) rows; reps=[z_i; z_j] (8192 x 256); sim = reps @ reps.T
  denom_r = sum_{c != r} exp(sim[r,c]/T);  pos_m = z_i[m].z_j[m]
  loss = mean_r( ln(denom_r) - pos_r/T )

Distribution (per sharding hint): core k receives ONLY its row shard
(emb_i rows [512k,512k+512) and emb_j rows likewise, fp16). It normalizes
its own 1024 reps rows, transposes them to d-major, AllGathers the
transposed fp16 reps across the 8 cores on-device (4MB), computes its
1024-row block of exp(sim/T) row-sums, and AllReduces the scalar-partial
[128,1] so every core holds the full-batch partial. The host fetches a
single 512B shard. Column order after the gather is a permutation of the
reference's reps order; row-wise denominators are permutation-invariant.

Per-core pipeline:
  - load own xa/xb fp16 [512,256] -> [128,4,256] tiles
  - rowwise sq-sums (DVE), inv_norm = Exp(-0.5*Ln(s)) (ACT), z = x*inv (fp16)
  - positives pos = (xa.xb)*inv_a*inv_b
  - DMA-xbar transpose own z -> zT halves [128d, 1024cols], store to DRAM
  - AllGather zT (fp16, 512KB->4MB) across 8 cores
  - per 2048-col group g: load rhs from gathered DRAM; per m-tile: matmul
    fp16 -> PSUM fp32 [128,2048], ACT Exp(scale=2) with accum_out row-sums
  - ln(rowsum - e^2) - 4*pos -> partial [128,1]; AllReduce add -> out
Host: loss = out_shard0.sum()/(2B).
"""

import os
import numpy as np
from contextlib import ExitStack

import concourse.bass as bass
import concourse.tile as tile
from concourse import bacc, mybir

B = 4096
D = 256
TEMP = 0.5
NCORES = 8
ROWS = 2 * B            # 8192 reps rows
PER = B // NCORES       # 512 rows of emb_i (and emb_j) per core
OWN = 2 * PER           # 1024 reps rows per core
P = 128
NG = 4                  # column groups
GCOLS = ROWS // NG      # 2048 columns per group
MT = OWN // P           # 8 m-tiles per core
F32 = mybir.dt.float32
DT = mybir.dt.float16   # compute/collective dtype
F8 = mybir.dt.float8e4  # host->device wire dtype (TRN E4M3, max +-240)
INV_T = 1.0 / TEMP      # 2.0
DIAG = float(np.exp(np.float32(INV_T), dtype=np.float32))  # exp(2*||z||^2), ||z||~1


def _kernel_body(ctx: ExitStack, tc: tile.TileContext, out_ap, x):
    nc = tc.nc
    AF = mybir.ActivationFunctionType
    ALU = mybir.AluOpType

    own_pool = ctx.enter_context(tc.tile_pool(name="own", bufs=1))
    sq_pool = ctx.enter_context(tc.tile_pool(name="sq", bufs=2))
    zt_pool = ctx.enter_context(tc.tile_pool(name="zt", bufs=1))
    fin_pool = ctx.enter_context(tc.tile_pool(name="fin", bufs=1))
    ps_pool = ctx.enter_context(tc.tile_pool(name="ps", bufs=2, space="PSUM"))
    dram = ctx.enter_context(tc.tile_pool(name="dram", bufs=1, space="DRAM"))

    rowparts = fin_pool.tile([P, MT * NG], F32, tag="rowparts")
    negdiag = fin_pool.tile([P, 1], F32, tag="negdiag")
    nc.gpsimd.memset(negdiag[:], -DIAG)

    # ---------------- own-block prologue ----------------
    # x rows: [own emb_i rows (512); own emb_j rows (512)] -> tiles 0-3 / 4-7
    nt_own = PER // P  # 4
    own_x8 = own_pool.tile([P, 2 * nt_own, D], F8, tag="own_x8")  # [128,8,256] fp8
    nc.sync.dma_start(own_x8[:], x.rearrange("(t p) d -> p t d", p=P))
    own_x = own_pool.tile([P, 2 * nt_own, D], DT, tag="own_x")
    nc.vector.tensor_copy(own_x[:], own_x8[:])

    sq3 = sq_pool.tile([P, 2 * nt_own, D], F32, tag="sq3", name="sq3")
    nc.vector.tensor_mul(sq3[:], own_x[:], own_x[:])
    sqs = own_pool.tile([P, 2 * nt_own], F32, tag="sqs")
    nc.vector.reduce_sum(out=sqs[:], in_=sq3[:], axis=mybir.AxisListType.X)
    inv = own_pool.tile([P, 2 * nt_own], F32, tag="inv")
    nc.scalar.activation(out=inv[:], in_=sqs[:], func=AF.Ln)
    nc.scalar.activation(out=inv[:], in_=inv[:], func=AF.Exp, scale=-0.5)

    z_own = own_pool.tile([P, 2 * nt_own, D], DT, tag="z_own")
    for t in range(2 * nt_own):
        nc.vector.tensor_scalar_mul(
            out=z_own[:, t, :], in0=own_x[:, t, :], scalar1=inv[:, t:t + 1])

    # positives: pos_t = (xa[t] . xb[t]) * inv_a[t] * inv_b[t]
    pr3 = sq_pool.tile([P, nt_own, D], F32, tag="sq3", name="pr3")
    nc.vector.tensor_mul(pr3[:], own_x[:, 0:nt_own, :], own_x[:, nt_own:2 * nt_own, :])
    pos_raw = own_pool.tile([P, nt_own], F32, tag="pos_raw")
    nc.vector.reduce_sum(out=pos_raw[:], in_=pr3[:], axis=mybir.AxisListType.X)
    pos = own_pool.tile([P, nt_own], F32, tag="pos")
    nc.vector.tensor_mul(pos[:], pos_raw[:], inv[:, 0:nt_own])
    nc.vector.tensor_mul(pos[:], pos[:], inv[:, nt_own:2 * nt_own])

    # transpose own z to d-major halves: zt_own[h][d, c] = z_own[c, :, h*128+d]
    zt_own = [own_pool.tile([P, OWN], DT, tag=f"zt_own{h}", name=f"zt_own{h}")
              for h in range(2)]
    for h in range(2):
        for t in range(2 * nt_own):
            nc.sync.dma_start_transpose(
                out=zt_own[h][:, t * P:(t + 1) * P],
                in_=z_own[:, t, h * P:(h + 1) * P])

    # ---------------- gather reps across cores ----------------
    ccin = dram.tile([2 * P, OWN], DT, tag="ccin", name="ccin")       # [256,1024]
    nc.sync.dma_start(ccin[0:P, :], zt_own[0][:])
    nc.sync.dma_start(ccin[P:2 * P, :], zt_own[1][:])
    ccout = dram.tile([NCORES * 2 * P, OWN], DT, tag="ccout", name="ccout")
    nc.gpsimd.collective_compute(
        "AllGather", ALU.bypass,
        replica_groups=[list(range(NCORES))],
        ins=[ccin[:].opt()], outs=[ccout[:].opt()])

    # rhs tiles: group g covers gathered cols of ranks 2g, 2g+1
    zt = [[None, None] for _ in range(NG)]
    for g in range(NG):
        for h in range(2):
            ztg = zt_pool.tile([P, GCOLS], DT, tag=f"zt{g}_{h}", name=f"zt{g}_{h}")
            for u in range(2):
                r = 2 * g + u
                nc.sync.dma_start(
                    ztg[:, u * OWN:(u + 1) * OWN],
                    ccout[r * 2 * P + h * P: r * 2 * P + (h + 1) * P, :])
            zt[g][h] = ztg

    # ---------------- main matmul + exp row-sums ----------------
    for g in range(NG):
        for m in range(MT):
            ps = ps_pool.tile([P, GCOLS], F32, tag="ps")
            nsub = GCOLS // 512
            for ns in range(nsub):
                nc.tensor.matmul(
                    ps[:, ns * 512:(ns + 1) * 512],
                    lhsT=zt_own[0][:, m * P:(m + 1) * P],
                    rhs=zt[g][0][:, ns * 512:(ns + 1) * 512],
                    start=True, stop=False)
            for ns in range(nsub):
                nc.tensor.matmul(
                    ps[:, ns * 512:(ns + 1) * 512],
                    lhsT=zt_own[1][:, m * P:(m + 1) * P],
                    rhs=zt[g][1][:, ns * 512:(ns + 1) * 512],
                    start=False, stop=True)
            nc.scalar.activation(
                out=ps[:], in_=ps[:], func=AF.Exp, scale=INV_T,
                accum_out=rowparts[:, m * NG + g: m * NG + g + 1])

    # ---------------- tail ----------------
    denom = fin_pool.tile([P, MT], F32, tag="denom")
    nc.vector.reduce_sum(
        out=denom[:], in_=rowparts[:].rearrange("p (m g) -> p m g", g=NG),
        axis=mybir.AxisListType.X)
    ln8 = fin_pool.tile([P, MT], F32, tag="ln8")
    nc.scalar.activation(out=ln8[:], in_=denom[:], func=AF.Ln, bias=negdiag[:])
    lnsum = fin_pool.tile([P, 1], F32, tag="lnsum")
    nc.vector.reduce_sum(out=lnsum[:], in_=ln8[:], axis=mybir.AxisListType.X)
    possum = fin_pool.tile([P, 1], F32, tag="possum")
    nc.vector.reduce_sum(out=possum[:], in_=pos[:], axis=mybir.AxisListType.X)
    partial = fin_pool.tile([P, 1], F32, tag="partial")
    # partial = lnsum - 2*INV_T*possum   (each pos appears for a z_i and a z_j row)
    nc.vector.tensor_scalar(
        out=partial[:], in0=possum[:], scalar1=-2.0 * INV_T, scalar2=lnsum[:],
        op0=ALU.mult, op1=ALU.add)

    # all-reduce the per-core partial so any single shard is the full answer
    ar_in = dram.tile([P, 1], F32, tag="ar_in", name="ar_in")
    ar_out = dram.tile([P, 1], F32, tag="ar_out", name="ar_out")
    nc.sync.dma_start(ar_in[:], partial[:])
    nc.gpsimd.collective_compute(
        "AllReduce", ALU.add,
        replica_groups=[list(range(NCORES))],
        ins=[ar_in[:].opt()], outs=[ar_out[:].opt()])
    nc.gpsimd.dma_start(out_ap, ar_out[:])


_NC_CACHE = {}


def build_nc():
    if "nc" in _NC_CACHE:
        return _NC_CACHE["nc"]
    nc = bacc.Bacc("TRN2", target_bir_lowering=False, debug=False,
                   enable_asserts=False, num_devices=NCORES)
    x = nc.dram_tensor("x", (OWN, D), F8, kind="ExternalInput").ap()
    out = nc.dram_tensor("out", (P, 1), F32, kind="ExternalOutput").ap()
    with tile.TileContext(nc) as tc:
        with ExitStack() as ctx:
            _kernel_body(ctx, tc, out, x)
    nc.compile()
    _NC_CACHE["nc"] = nc
    return nc


def pack_inputs(emb_i, emb_j):
    """[8192,256] fp8-e4m3: per core k, its 512 emb_i rows then its 512 emb_j rows."""
    import ml_dtypes
    combined = np.empty((NCORES, 2, PER, D), ml_dtypes.float8_e4m3)
    np.copyto(combined[:, 0], np.asarray(emb_i).reshape(NCORES, PER, D),
              casting="unsafe")
    np.copyto(combined[:, 1], np.asarray(emb_j).reshape(NCORES, PER, D),
              casting="unsafe")
    return combined.reshape(ROWS, D)


def make_in_maps(x_global):
    return [{"x": x_global[k * OWN:(k + 1) * OWN]} for k in range(NCORES)]


# ---------------- cached PJRT dispatcher ----------------
# run_bass_kernel_spmd rebuilds jit(shard_map(...)) on every call (fresh
# closure -> jit cache miss -> full retrace each run). Build it once and
# reuse; identical execution path (same _bass_exec_p custom call, same NEFF,
# cores 0-7), minus the per-call retrace.

_DISP = {}


def _dispatcher():
    if "d" in _DISP:
        return _DISP["d"]
    import jax
    from jax.sharding import Mesh, PartitionSpec
    try:
        from jax.experimental.shard_map import shard_map  # what bass2jax uses
        sm_kw = {"check_rep": False}
    except ImportError:
        from jax import shard_map
        sm_kw = {"check_vma": False}
    from concourse.bass2jax import (
        _bass_exec_p, install_neuronx_cc_hook, partition_id_tensor)

    nc = build_nc()
    install_neuronx_cc_hook()

    partition_name = nc.partition_id_tensor.name if nc.partition_id_tensor else None
    in_names, out_names, out_avals = [], [], []
    for alloc in nc.m.functions[0].allocations:
        if not isinstance(alloc, mybir.MemoryLocationSet):
            continue
        name = alloc.memorylocations[0].name
        if alloc.kind == "ExternalInput":
            if name != partition_name:
                in_names.append(name)
        elif alloc.kind == "ExternalOutput":
            shape = tuple(alloc.tensor_shape)
            dtype = mybir.dt.np(alloc.dtype)
            out_names.append(name)
            out_avals.append(jax.core.ShapedArray(shape, dtype))
    n_params = len(in_names)
    n_outs = len(out_names)
    # No donated zero-output operands: this kernel writes every element of
    # every ExternalOutput, so uninitialized PJRT-allocated results are fine.
    in_names_all = list(in_names)
    if partition_name is not None:
        in_names_all.append(partition_name)

    def _body(*args):
        operands = list(args)
        if partition_name is not None:
            operands.append(partition_id_tensor())
        outs = _bass_exec_p.bind(
            *operands,
            out_avals=tuple(out_avals),
            in_names=tuple(in_names_all),
            out_names=tuple(out_names),
            lowering_input_output_aliases=(),
            sim_require_finite=True,
            sim_require_nnan=True,
            nc=nc,
        )
        return tuple(outs)

    devices = jax.devices()[:NCORES]
    assert len(devices) == NCORES
    mesh = Mesh(np.asarray(devices), ("core",))
    in_specs = (PartitionSpec("core"),) * n_params
    out_specs = (PartitionSpec("core"),) * n_outs
    sharded = jax.jit(
        shard_map(_body, mesh=mesh, in_specs=in_specs, out_specs=out_specs,
                  **sm_kw))
    d = {"sharded": sharded, "in_names": in_names, "out_names": out_names}
    _DISP["d"] = d
    return d


def run_cached(x_global):
    """One SPMD run via the cached dispatcher; returns core-0's out shard."""
    d = _dispatcher()
    out_arrs = d["sharded"](x_global)
    try:
        # single-shard fetch: out is AllReduced, any core's [128,1] is the answer
        return np.asarray(out_arrs[0].addressable_shards[0].data)
    except Exception:
        return np.asarray(out_arrs[0])[:P]


def run_spmd(x_global):
    """Fallback: same NEFF via bass_utils.run_bass_kernel_spmd."""
    from concourse import bass_utils
    nc = build_nc()
    res = bass_utils.run_bass_kernel_spmd(
        nc, make_in_maps(x_global), core_ids=list(range(NCORES)))
    return np.asarray(res.results[0]["out"])


def kernel(emb_i, emb_j):
    x_global = pack_inputs(emb_i, emb_j)
    if os.environ.get("CL_DISPATCH", "cached") == "spmd":
        part = run_spmd(x_global)
    else:
        part = run_cached(x_global)
    loss = np.float32(part.astype(np.float64).sum() / ROWS)
    return np.asarray(loss, dtype=np.float32)


# revision 16
# speedup vs baseline: 1.5676x; 1.0390x over previous
"""Contrastive (NT-Xent) loss kernel for Trainium2, 8 NeuronCores SPMD.

Math (B=4096, D=256, T=0.5):
  z = l2norm(emb) rows; reps=[z_i; z_j] (8192 x 256); sim = reps @ reps.T
  denom_r = sum_{c != r} exp(sim[r,c]/T);  pos_m = z_i[m].z_j[m]
  loss = mean_r( ln(denom_r) - pos_r/T )

Distribution (per sharding hint): core k receives ONLY its row shard
(emb_i rows [512k,512k+512) and emb_j rows likewise, fp16). It normalizes
its own 1024 reps rows, transposes them to d-major, AllGathers the
transposed fp16 reps across the 8 cores on-device (4MB), computes its
1024-row block of exp(sim/T) row-sums, and AllReduces the scalar-partial
[128,1] so every core holds the full-batch partial. The host fetches a
single 512B shard. Column order after the gather is a permutation of the
reference's reps order; row-wise denominators are permutation-invariant.

Per-core pipeline:
  - load own xa/xb fp16 [512,256] -> [128,4,256] tiles
  - rowwise sq-sums (DVE), inv_norm = Exp(-0.5*Ln(s)) (ACT), z = x*inv (fp16)
  - positives pos = (xa.xb)*inv_a*inv_b
  - DMA-xbar transpose own z -> zT halves [128d, 1024cols], store to DRAM
  - AllGather zT (fp16, 512KB->4MB) across 8 cores
  - per 2048-col group g: load rhs from gathered DRAM; per m-tile: matmul
    fp16 -> PSUM fp32 [128,2048], ACT Exp(scale=2) with accum_out row-sums
  - ln(rowsum - e^2) - 4*pos -> partial [128,1]; AllReduce add -> out
Host: loss = out_shard0.sum()/(2B).
"""

import os
import numpy as np
from contextlib import ExitStack

import concourse.bass as bass
import concourse.tile as tile
from concourse import bacc, mybir

B = 4096
D = 256
TEMP = 0.5
NCORES = 8
ROWS = 2 * B            # 8192 reps rows
PER = B // NCORES       # 512 rows of emb_i (and emb_j) per core
OWN = 2 * PER           # 1024 reps rows per core
P = 128
NG = 4                  # column groups
GCOLS = ROWS // NG      # 2048 columns per group
MT = OWN // P           # 8 m-tiles per core
F32 = mybir.dt.float32
DT = mybir.dt.float16   # compute/collective dtype
F8 = mybir.dt.float8e4  # host->device wire dtype (TRN E4M3, max +-240)
INV_T = 1.0 / TEMP      # 2.0
DIAG = float(np.exp(np.float32(INV_T), dtype=np.float32))  # exp(2*||z||^2), ||z||~1


def _kernel_body(ctx: ExitStack, tc: tile.TileContext, out_ap, x):
    nc = tc.nc
    AF = mybir.ActivationFunctionType
    ALU = mybir.AluOpType

    own_pool = ctx.enter_context(tc.tile_pool(name="own", bufs=1))
    sq_pool = ctx.enter_context(tc.tile_pool(name="sq", bufs=2))
    zt_pool = ctx.enter_context(tc.tile_pool(name="zt", bufs=1))
    fin_pool = ctx.enter_context(tc.tile_pool(name="fin", bufs=1))
    ps_pool = ctx.enter_context(tc.tile_pool(name="ps", bufs=2, space="PSUM"))
    dram = ctx.enter_context(tc.tile_pool(name="dram", bufs=1, space="DRAM"))

    rowparts = fin_pool.tile([P, MT * NG], F32, tag="rowparts")
    negdiag = fin_pool.tile([P, 1], F32, tag="negdiag")
    nc.gpsimd.memset(negdiag[:], -DIAG)

    # ---------------- own-block prologue ----------------
    # x rows: [own emb_i rows (512); own emb_j rows (512)] -> tiles 0-3 / 4-7
    nt_own = PER // P  # 4
    own_x8 = own_pool.tile([P, 2 * nt_own, D], F8, tag="own_x8")  # [128,8,256] fp8
    nc.sync.dma_start(own_x8[:], x.rearrange("(t p) d -> p t d", p=P))
    own_x = own_pool.tile([P, 2 * nt_own, D], DT, tag="own_x")
    nc.vector.tensor_copy(own_x[:], own_x8[:])

    sq3 = sq_pool.tile([P, 2 * nt_own, D], F32, tag="sq3", name="sq3")
    nc.vector.tensor_mul(sq3[:], own_x[:], own_x[:])
    sqs = own_pool.tile([P, 2 * nt_own], F32, tag="sqs")
    nc.vector.reduce_sum(out=sqs[:], in_=sq3[:], axis=mybir.AxisListType.X)
    inv = own_pool.tile([P, 2 * nt_own], F32, tag="inv")
    nc.scalar.activation(out=inv[:], in_=sqs[:], func=AF.Ln)
    nc.scalar.activation(out=inv[:], in_=inv[:], func=AF.Exp, scale=-0.5)

    z_own = own_pool.tile([P, 2 * nt_own, D], DT, tag="z_own")
    for t in range(2 * nt_own):
        nc.vector.tensor_scalar_mul(
            out=z_own[:, t, :], in0=own_x[:, t, :], scalar1=inv[:, t:t + 1])

    # positives: pos_t = (xa[t] . xb[t]) * inv_a[t] * inv_b[t]
    pr3 = sq_pool.tile([P, nt_own, D], F32, tag="sq3", name="pr3")
    nc.vector.tensor_mul(pr3[:], own_x[:, 0:nt_own, :], own_x[:, nt_own:2 * nt_own, :])
    pos_raw = own_pool.tile([P, nt_own], F32, tag="pos_raw")
    nc.vector.reduce_sum(out=pos_raw[:], in_=pr3[:], axis=mybir.AxisListType.X)
    pos = own_pool.tile([P, nt_own], F32, tag="pos")
    nc.vector.tensor_mul(pos[:], pos_raw[:], inv[:, 0:nt_own])
    nc.vector.tensor_mul(pos[:], pos[:], inv[:, nt_own:2 * nt_own])

    # transpose own z to d-major halves: zt_own[h][d, c] = z_own[c, :, h*128+d]
    zt_own = [own_pool.tile([P, OWN], DT, tag=f"zt_own{h}", name=f"zt_own{h}")
              for h in range(2)]
    for h in range(2):
        for t in range(2 * nt_own):
            nc.sync.dma_start_transpose(
                out=zt_own[h][:, t * P:(t + 1) * P],
                in_=z_own[:, t, h * P:(h + 1) * P])

    # ---------------- gather reps across cores ----------------
    ccin = dram.tile([2 * P, OWN], DT, tag="ccin", name="ccin")       # [256,1024]
    nc.sync.dma_start(ccin[0:P, :], zt_own[0][:])
    nc.sync.dma_start(ccin[P:2 * P, :], zt_own[1][:])
    ccout = dram.tile([NCORES * 2 * P, OWN], DT, tag="ccout", name="ccout")
    nc.gpsimd.collective_compute(
        "AllGather", ALU.bypass,
        replica_groups=[list(range(NCORES))],
        ins=[ccin[:].opt()], outs=[ccout[:].opt()])

    # rhs tiles: group g covers gathered cols of ranks 2g, 2g+1
    zt = [[None, None] for _ in range(NG)]
    for g in range(NG):
        for h in range(2):
            ztg = zt_pool.tile([P, GCOLS], DT, tag=f"zt{g}_{h}", name=f"zt{g}_{h}")
            for u in range(2):
                r = 2 * g + u
                nc.sync.dma_start(
                    ztg[:, u * OWN:(u + 1) * OWN],
                    ccout[r * 2 * P + h * P: r * 2 * P + (h + 1) * P, :])
            zt[g][h] = ztg

    # ---------------- main matmul + exp row-sums ----------------
    for g in range(NG):
        for m in range(MT):
            ps = ps_pool.tile([P, GCOLS], F32, tag="ps")
            nsub = GCOLS // 512
            for ns in range(nsub):
                nc.tensor.matmul(
                    ps[:, ns * 512:(ns + 1) * 512],
                    lhsT=zt_own[0][:, m * P:(m + 1) * P],
                    rhs=zt[g][0][:, ns * 512:(ns + 1) * 512],
                    start=True, stop=False)
            for ns in range(nsub):
                nc.tensor.matmul(
                    ps[:, ns * 512:(ns + 1) * 512],
                    lhsT=zt_own[1][:, m * P:(m + 1) * P],
                    rhs=zt[g][1][:, ns * 512:(ns + 1) * 512],
                    start=False, stop=True)
            nc.scalar.activation(
                out=ps[:], in_=ps[:], func=AF.Exp, scale=INV_T,
                accum_out=rowparts[:, m * NG + g: m * NG + g + 1])

    # ---------------- tail ----------------
    denom = fin_pool.tile([P, MT], F32, tag="denom")
    nc.vector.reduce_sum(
        out=denom[:], in_=rowparts[:].rearrange("p (m g) -> p m g", g=NG),
        axis=mybir.AxisListType.X)
    ln8 = fin_pool.tile([P, MT], F32, tag="ln8")
    nc.scalar.activation(out=ln8[:], in_=denom[:], func=AF.Ln, bias=negdiag[:])
    lnsum = fin_pool.tile([P, 1], F32, tag="lnsum")
    nc.vector.reduce_sum(out=lnsum[:], in_=ln8[:], axis=mybir.AxisListType.X)
    possum = fin_pool.tile([P, 1], F32, tag="possum")
    nc.vector.reduce_sum(out=possum[:], in_=pos[:], axis=mybir.AxisListType.X)
    partial = fin_pool.tile([P, 1], F32, tag="partial")
    # partial = lnsum - 2*INV_T*possum   (each pos appears for a z_i and a z_j row)
    nc.vector.tensor_scalar(
        out=partial[:], in0=possum[:], scalar1=-2.0 * INV_T, scalar2=lnsum[:],
        op0=ALU.mult, op1=ALU.add)

    # all-reduce the per-core partial so any single shard is the full answer
    ar_in = dram.tile([P, 1], F32, tag="ar_in", name="ar_in")
    ar_out = dram.tile([P, 1], F32, tag="ar_out", name="ar_out")
    nc.sync.dma_start(ar_in[:], partial[:])
    nc.gpsimd.collective_compute(
        "AllReduce", ALU.add,
        replica_groups=[list(range(NCORES))],
        ins=[ar_in[:].opt()], outs=[ar_out[:].opt()])
    nc.gpsimd.dma_start(out_ap, ar_out[:])


_NC_CACHE = {}


def build_nc():
    if "nc" in _NC_CACHE:
        return _NC_CACHE["nc"]
    nc = bacc.Bacc("TRN2", target_bir_lowering=False, debug=False,
                   enable_asserts=False, num_devices=NCORES)
    x = nc.dram_tensor("x", (OWN, D), F8, kind="ExternalInput").ap()
    out = nc.dram_tensor("out", (P, 1), F32, kind="ExternalOutput").ap()
    with tile.TileContext(nc) as tc:
        with ExitStack() as ctx:
            _kernel_body(ctx, tc, out, x)
    nc.compile()
    _NC_CACHE["nc"] = nc
    return nc


_PACK = {}


def _pack_numpy(emb_i, emb_j):
    import ml_dtypes
    combined = np.empty((NCORES, 2, PER, D), ml_dtypes.float8_e4m3)
    np.copyto(combined[:, 0], np.asarray(emb_i).reshape(NCORES, PER, D),
              casting="unsafe")
    np.copyto(combined[:, 1], np.asarray(emb_j).reshape(NCORES, PER, D),
              casting="unsafe")
    return combined.reshape(ROWS, D)


def pack_inputs(emb_i, emb_j):
    """[8192,256] fp8-e4m3: per core k, its 512 emb_i rows then its 512 emb_j rows."""
    emb_i = np.asarray(emb_i, dtype=np.float32)
    emb_j = np.asarray(emb_j, dtype=np.float32)
    try:
        import jax
        import jax.numpy as jnp
        if "fn" not in _PACK:
            def _pack_xla(a, b):
                a = a.reshape(NCORES, PER, D).astype(jnp.float8_e4m3)
                b = b.reshape(NCORES, PER, D).astype(jnp.float8_e4m3)
                return jnp.stack([a, b], axis=1).reshape(ROWS, D)
            _PACK["fn"] = jax.jit(_pack_xla)
            _PACK["cpu"] = jax.devices("cpu")[0]
        with jax.default_device(_PACK["cpu"]):
            return np.asarray(_PACK["fn"](emb_i, emb_j))
    except Exception:
        return _pack_numpy(emb_i, emb_j)


def make_in_maps(x_global):
    return [{"x": x_global[k * OWN:(k + 1) * OWN]} for k in range(NCORES)]


# ---------------- cached PJRT dispatcher ----------------
# run_bass_kernel_spmd rebuilds jit(shard_map(...)) on every call (fresh
# closure -> jit cache miss -> full retrace each run). Build it once and
# reuse; identical execution path (same _bass_exec_p custom call, same NEFF,
# cores 0-7), minus the per-call retrace.

_DISP = {}


def _dispatcher():
    if "d" in _DISP:
        return _DISP["d"]
    import jax
    from jax.sharding import Mesh, PartitionSpec
    try:
        from jax.experimental.shard_map import shard_map  # what bass2jax uses
        sm_kw = {"check_rep": False}
    except ImportError:
        from jax import shard_map
        sm_kw = {"check_vma": False}
    from concourse.bass2jax import (
        _bass_exec_p, install_neuronx_cc_hook, partition_id_tensor)

    nc = build_nc()
    install_neuronx_cc_hook()

    partition_name = nc.partition_id_tensor.name if nc.partition_id_tensor else None
    in_names, out_names, out_avals = [], [], []
    for alloc in nc.m.functions[0].allocations:
        if not isinstance(alloc, mybir.MemoryLocationSet):
            continue
        name = alloc.memorylocations[0].name
        if alloc.kind == "ExternalInput":
            if name != partition_name:
                in_names.append(name)
        elif alloc.kind == "ExternalOutput":
            shape = tuple(alloc.tensor_shape)
            dtype = mybir.dt.np(alloc.dtype)
            out_names.append(name)
            out_avals.append(jax.core.ShapedArray(shape, dtype))
    n_params = len(in_names)
    n_outs = len(out_names)
    # No donated zero-output operands: this kernel writes every element of
    # every ExternalOutput, so uninitialized PJRT-allocated results are fine.
    in_names_all = list(in_names)
    if partition_name is not None:
        in_names_all.append(partition_name)

    def _body(*args):
        operands = list(args)
        if partition_name is not None:
            operands.append(partition_id_tensor())
        outs = _bass_exec_p.bind(
            *operands,
            out_avals=tuple(out_avals),
            in_names=tuple(in_names_all),
            out_names=tuple(out_names),
            lowering_input_output_aliases=(),
            sim_require_finite=True,
            sim_require_nnan=True,
            nc=nc,
        )
        return tuple(outs)

    devices = jax.devices()[:NCORES]
    assert len(devices) == NCORES
    mesh = Mesh(np.asarray(devices), ("core",))
    in_specs = (PartitionSpec("core"),) * n_params
    out_specs = (PartitionSpec("core"),) * n_outs
    sharded = jax.jit(
        shard_map(_body, mesh=mesh, in_specs=in_specs, out_specs=out_specs,
                  **sm_kw))
    d = {"sharded": sharded, "in_names": in_names, "out_names": out_names}
    _DISP["d"] = d
    return d


def run_cached(x_global):
    """One SPMD run via the cached dispatcher; returns core-0's out shard."""
    d = _dispatcher()
    out_arrs = d["sharded"](x_global)
    try:
        # single-shard fetch: out is AllReduced, any core's [128,1] is the answer
        return np.asarray(out_arrs[0].addressable_shards[0].data)
    except Exception:
        return np.asarray(out_arrs[0])[:P]


def run_spmd(x_global):
    """Fallback: same NEFF via bass_utils.run_bass_kernel_spmd."""
    from concourse import bass_utils
    nc = build_nc()
    res = bass_utils.run_bass_kernel_spmd(
        nc, make_in_maps(x_global), core_ids=list(range(NCORES)))
    return np.asarray(res.results[0]["out"])


def kernel(emb_i, emb_j):
    x_global = pack_inputs(emb_i, emb_j)
    if os.environ.get("CL_DISPATCH", "cached") == "spmd":
        part = run_spmd(x_global)
    else:
        part = run_cached(x_global)
    loss = np.float32(part.astype(np.float64).sum() / ROWS)
    return np.asarray(loss, dtype=np.float32)
